# revision 1
# baseline (speedup 1.0000x reference)
"""HMM log-likelihood (log-domain forward algorithm) on 8 Trainium2 cores.

Strategy: scaled linear-domain forward algorithm with warmup-halo sequence
parallelism.  The filtering distribution of an HMM forgets its initial
condition geometrically fast, so N=1e6 timesteps are split into 3840
independent chains (480/core); each chain starts from a uniform state W=20
steps before its owned region of L=260 steps.  Per core, chains are batched
4-wide across the 128 SBUF partitions (block-diagonal T^T weights on the PE)
with the chain-block index in the matmul free dimension, so each timestep is
one bf16 matmul (T @ S into PSUM) plus one vector multiply by the emission
probabilities.

Normalization is free: a constant per-step drift delta = E[log c] is folded
into the exp bias, making log|S| a zero-drift random walk (~26 bits 4.5
sigma over a 280-step chain — far inside f32 range), so the kernel needs no
per-chain rescaling.  The bf16 quantization of T factors exactly as
D_r @ T_hat with T_hat row-stochastic; -log(r) is folded into the same exp
bias.  Each chain's contribution is log(sum(S_final)) - log(sum(S_at_W)) +
delta*L, assembled on the host, which also runs exact f64 scans for the
prefix [0, W) and the short tail.
"""

import sys

for p in ("/opt/trn_rl_repo", "/root/.axon_site", "/root/.axon_site/_ro/trn_rl_repo",
          "/root/.axon_site/_ro/pypackages"):
    if p not in sys.path:
        sys.path.insert(0, p)

import numpy as np

K = 32
N = 1_000_000
NCORES = 8
W = 20            # warmup (halo) steps per chain
L = 260           # owned steps per chain
CC = 480          # chains per core
SPAN = W + L      # 280 sequential steps
SBLK = 140        # timesteps per load window
NWIN = SPAN // SBLK
NB = CC // 4      # 120 four-chain blocks
G = 2             # interleaved compute groups
F = NB // G       # 60 blocks (matmul free dim) per group
NSL = CC * L + W  # per-core input slice columns
COVERED = W + NCORES * CC * L

_cache = {}


def _build():
    import concourse.bass as bass
    import concourse.bacc as bacc
    import concourse.mybir as mybir
    import concourse.tile as tile
    from contextlib import ExitStack

    f32 = mybir.dt.float32
    bf16 = mybir.dt.bfloat16
    AF = mybir.ActivationFunctionType

    nc = bacc.Bacc("TRN2", target_bir_lowering=False, debug=False,
                   num_devices=NCORES)
    x = nc.dram_tensor("x", [K, NSL], f32, kind="ExternalInput")
    wmat = nc.dram_tensor("wmat", [128, 128], bf16, kind="ExternalInput")
    ebias = nc.dram_tensor("ebias", [128, 1], f32, kind="ExternalInput")
    snap_out = nc.dram_tensor("snap_out", [128, NB], bf16, kind="ExternalOutput")
    fin_out = nc.dram_tensor("fin_out", [128, NB], bf16, kind="ExternalOutput")

    with tile.TileContext(nc) as tc:
        with ExitStack() as ctx:
            cpool = ctx.enter_context(tc.tile_pool(name="const", bufs=1))
            rpool = ctx.enter_context(tc.tile_pool(name="rp", bufs=NWIN))
            pspool = ctx.enter_context(
                tc.tile_pool(name="ps", bufs=2, space=bass.MemorySpace.PSUM))

            w_t = cpool.tile([128, 128], bf16, tag="w")
            nc.sync.dma_start(w_t[:], wmat[:])
            eb_t = cpool.tile([128, 1], f32, tag="eb")
            nc.sync.dma_start(eb_t[:], ebias[:])

            spool = ctx.enter_context(tc.tile_pool(name="sp", bufs=2))
            S, SN = [], []
            for g in range(G):
                st = spool.tile([128, F], bf16, tag=f"S{g}", name=f"st{g}")
                nc.vector.memset(st[:], 1.0)
                sn = cpool.tile([128, F], bf16, tag=f"N{g}")
                S.append(st)
                SN.append(sn)

            # Load + exp windows.  R[g][w] layout: [128, F, SBLK], partition
            # p = 32*q + k holds chain (g*F + cb)*4 + q, state k.
            R = [[None] * NWIN for _ in range(G)]
            NCHUNK = 4
            CH = F // NCHUNK
            # interleave DMA chunks and exp chunks across groups so both
            # chains become runnable at the same (early) time
            for w in range(NWIN):
                for g in range(G):
                    rt = rpool.tile([128, F, SBLK], f32, tag=f"R{g}",
                                    name=f"rt{g}_{w}")
                    R[g][w] = rt
                for ch in range(NCHUNK):
                    for g in range(G):
                        rt = R[g][w]
                        cb0 = ch * CH
                        for q in range(4):
                            off = ((g * F + cb0) * 4 + q) * L + w * SBLK
                            src = bass.AP(x, off,
                                          [[NSL, 32], [4 * L, CH], [1, SBLK]])
                            nc.sync.dma_start(
                                rt[32 * q:32 * q + 32, cb0:cb0 + CH, :], src)
                # exp in place, chunked along s so compute starts early
                EC = 7
                for ec in range(EC):
                    for g in range(G):
                        rt = R[g][w]
                        s0 = ec * (SBLK // EC)
                        nc.scalar.activation(
                            rt[:, :, s0:s0 + SBLK // EC],
                            rt[:, :, s0:s0 + SBLK // EC], AF.Exp,
                            bias=eb_t[:])

            for s in range(SPAN):
                w, si = divmod(s, SBLK)
                for g in range(G):
                    ps = pspool.tile([128, F], f32, tag=f"mm{g}")
                    nc.tensor.matmul(ps[:], w_t[:], S[g][:], start=True, stop=True)
                    # ping-pong the state tile so the multiply never WARs
                    # against this step's matmul read
                    sn_new = spool.tile([128, F], bf16, tag=f"S{g}",
                                        name=f"st{g}_{s}")
                    nc.vector.tensor_mul(sn_new[:], ps[:], R[g][w][:, :, si])
                    S[g] = sn_new
                    if s == W - 1:
                        nc.vector.tensor_copy(SN[g][:], S[g][:])

            for g in range(G):
                nc.sync.dma_start(snap_out[:, g * F:(g + 1) * F], SN[g][:])
                nc.sync.dma_start(fin_out[:, g * F:(g + 1) * F], S[g][:])

    nc.compile()
    return nc


def _get_nc():
    if "nc" not in _cache:
        _cache["nc"] = _build()
    return _cache["nc"]


def _log_softmax64(v, axis):
    v = v.astype(np.float64)
    m = v.max(axis=axis, keepdims=True)
    e = np.exp(v - m)
    return v - m - np.log(e.sum(axis=axis, keepdims=True))


def _estimate_delta(log_pdf, T64):
    # E[log c] from a vectorized short scan: 64 parallel probes, 56 steps,
    # burn-in 16 (mixing time is ~10 steps).
    NCH, NST, BURN = 64, 56, 16
    cols = np.arange(NCH) * 997 + 1
    a = np.full((K, NCH), 1.0 / K)
    samples = []
    for s in range(NST):
        p = np.exp(log_pdf[:, cols + s].astype(np.float64))
        a = p * (T64 @ a)
        c = a.sum(axis=0)
        a /= c
        if s >= BURN:
            samples.append(np.log(c))
    return float(np.mean(samples))


def _make_in_maps(log_pdf, T64):
    from ml_dtypes import bfloat16

    T32 = T64.astype(np.float32)
    Tbf = T32.astype(bfloat16)
    delta = _estimate_delta(log_pdf, T64)
    # bf16-quantized T is exactly D_r @ T_hat with T_hat row-stochastic and
    # r the bf16 row sums; fold -log(r) and the drift -delta into the exp.
    r = Tbf.astype(np.float64).sum(axis=1)
    eb = np.zeros((128, 1), dtype=np.float32)
    for q in range(4):
        eb[32 * q:32 * q + 32, 0] = (-np.log(r) - delta).astype(np.float32)
    wm = np.zeros((128, 128), dtype=bfloat16)
    for q in range(4):
        wm[32 * q:32 * q + 32, 32 * q:32 * q + 32] = Tbf.T
    in_maps = []
    for k in range(NCORES):
        c0 = k * CC * L
        in_maps.append({
            "x": np.ascontiguousarray(log_pdf[:, c0:c0 + NSL]),
            "wmat": wm,
            "ebias": eb,
        })

    return in_maps, delta


def kernel(log_pdf: np.ndarray, pi: np.ndarray, T: np.ndarray) -> np.ndarray:
    from concourse.bass_utils import run_bass_kernel_spmd

    log_pdf = np.ascontiguousarray(log_pdf, dtype=np.float32)
    log_pi64 = _log_softmax64(pi, 0)
    log_T64 = _log_softmax64(T, 1)
    T64 = np.exp(log_T64)                     # row-stochastic [K, K] f64

    in_maps, delta = _make_in_maps(log_pdf, T64)
    nc = _get_nc()
    res = run_bass_kernel_spmd(nc, in_maps, list(range(NCORES))).results

    # ---- host combine (f64) ----
    LP = log_pdf
    # exact prefix [0, W)
    a = np.exp(log_pi64 + LP[:, 0].astype(np.float64))
    c = a.sum()
    total = np.log(c)
    a /= c
    for t in range(1, W):
        a = np.exp(LP[:, t].astype(np.float64)) * (T64 @ a)
        c = a.sum()
        total += np.log(c)
        a /= c

    # per-chain contributions: log(sum fin) - log(sum snap) + delta*L
    for k in range(NCORES):
        snap = res[k]["snap_out"].astype(np.float64)   # [128, NB]
        fin = res[k]["fin_out"].astype(np.float64)
        for q in range(4):
            ssum = snap[32 * q:32 * q + 32, :].sum(axis=0)
            fsum = fin[32 * q:32 * q + 32, :].sum(axis=0)
            total += (np.log(fsum) - np.log(ssum)).sum() + delta * L * NB

    # exact tail [COVERED, N) from the last chain's final state
    k, g, cb, q = NCORES - 1, G - 1, F - 1, 3
    fv = res[k]["fin_out"][32 * q:32 * q + 32, g * F + cb].astype(np.float64)
    a = fv / fv.sum()
    for t in range(COVERED, N):
        a = np.exp(LP[:, t].astype(np.float64)) * (T64 @ a)
        c = a.sum()
        total += np.log(c)
        a /= c

    return np.float32(total)



# revision 2
# speedup vs baseline: 3.0382x; 3.0382x over previous
"""HMM log-likelihood (log-domain forward algorithm) on 8 Trainium2 cores.

Strategy: scaled linear-domain forward algorithm with warmup-halo sequence
parallelism.  The filtering distribution of an HMM forgets its initial
condition geometrically fast (|lambda_2(T)| ~ 0.24), so N=1e6 timesteps are
split into short independent chains; each chain starts from a uniform state
W steps before its owned region of L steps.  Per core, chains are batched
4-wide across the 128 SBUF partitions (block-diagonal T^T weights on the PE)
with the chain-block index in the matmul free dimension, so one timestep is
G matmuls (T @ S into PSUM) plus G vector multiplies by the emission
probabilities.

The host pre-packs the per-core input into the exact SBUF tile layout
[128 partitions, SPAN, NB] (time-major), so every DMA is a fully
contiguous [128, chunk] copy at near-peak HBM bandwidth, and every
per-step emission slice is a contiguous [128, F] access.

Normalization is free: a constant per-step drift delta = E[log c] is folded
into the exp bias, making log|S| a zero-drift random walk, so the kernel
needs no per-chain rescaling.  The bf16 quantization of T factors exactly as
D_r @ T_hat with T_hat row-stochastic; -log(r) is folded into the same exp
bias.  Each chain's contribution is log(sum(S_final)) - log(sum(S_at_W)) +
delta*L, assembled on the host, which also runs exact f64 scans for the
prefix [0, W) and the short tail.
"""

import sys

for p in ("/opt/trn_rl_repo", "/root/.axon_site", "/root/.axon_site/_ro/trn_rl_repo",
          "/root/.axon_site/_ro/pypackages"):
    if p not in sys.path:
        sys.path.insert(0, p)

import numpy as np

K = 32
N = 1_000_000
NCORES = 8
W = 5             # warmup (halo) steps per chain
L = 31            # owned steps per chain
SPAN = W + L      # 36 sequential steps
CC = 4032         # chains per core (owned columns = CC*L = 124992)
NB = CC // 4      # 1008 four-chain blocks (matmul/mul free dim)
G = 2             # interleaved compute streams
F = NB // G       # 504 blocks per stream (fits one PSUM bank in f32)
SBLK = 4          # timesteps per load window
NWIN = SPAN // SBLK  # 9 windows
COVERED = W + NCORES * CC * L

_cache = {}


def _build():
    import concourse.bass as bass
    import concourse.bacc as bacc
    import concourse.mybir as mybir
    import concourse.tile as tile
    from contextlib import ExitStack

    f32 = mybir.dt.float32
    bf16 = mybir.dt.bfloat16
    AF = mybir.ActivationFunctionType

    M = SPAN * NB  # per-partition input columns

    nc = bacc.Bacc("TRN2", target_bir_lowering=False, debug=False,
                   num_devices=NCORES)
    x = nc.dram_tensor("x", [128, M], f32, kind="ExternalInput")
    wmat = nc.dram_tensor("wmat", [128, 128], bf16, kind="ExternalInput")
    ebias = nc.dram_tensor("ebias", [128, 1], f32, kind="ExternalInput")
    snap_out = nc.dram_tensor("snap_out", [128, NB], bf16, kind="ExternalOutput")
    fin_out = nc.dram_tensor("fin_out", [128, NB], bf16, kind="ExternalOutput")

    with tile.TileContext(nc) as tc:
        with ExitStack() as ctx:
            cpool = ctx.enter_context(tc.tile_pool(name="const", bufs=1))
            rpool = ctx.enter_context(tc.tile_pool(name="rp", bufs=3))
            pspool = ctx.enter_context(
                tc.tile_pool(name="ps", bufs=4, space=bass.MemorySpace.PSUM))

            w_t = cpool.tile([128, 128], bf16, tag="w")
            nc.sync.dma_start(w_t[:], wmat[:])
            eb_t = cpool.tile([128, 1], f32, tag="eb")
            nc.sync.dma_start(eb_t[:], ebias[:])

            spool = ctx.enter_context(tc.tile_pool(name="sp", bufs=2))
            S, SN = [], []
            for g in range(G):
                st = spool.tile([128, F], bf16, tag=f"S{g}", name=f"st{g}")
                nc.vector.memset(st[:], 1.0)
                sn = cpool.tile([128, F], bf16, tag=f"N{g}")
                S.append(st)
                SN.append(sn)

            # Load + exp windows.  R[w] layout: [128, SBLK, NB]; partition
            # p = 32*q + k holds chain 4*cb + q, state k at free (s, cb).
            # The host packs x so each window is one contiguous DMA.
            R = [None] * NWIN
            EC = 2  # exp chunks per window (along s)
            for w in range(NWIN):
                rt = rpool.tile([128, SBLK, NB], f32, tag="R", name=f"rt{w}")
                src = bass.AP(x, w * SBLK * NB,
                              [[M, 128], [NB, SBLK], [1, NB]])
                nc.sync.dma_start(rt[:], src)
                R[w] = rt
                cs = SBLK // EC
                for ec in range(EC):
                    s0 = ec * cs
                    nc.scalar.activation(
                        rt[:, s0:s0 + cs, :], rt[:, s0:s0 + cs, :], AF.Exp,
                        bias=eb_t[:])

            for s in range(SPAN):
                w, si = divmod(s, SBLK)
                for g in range(G):
                    ps = pspool.tile([128, F], f32, tag=f"mm{g}")
                    nc.tensor.matmul(ps[:], w_t[:], S[g][:], start=True, stop=True)
                    # ping-pong the state tile so the multiply never WARs
                    # against this step's matmul read
                    sn_new = spool.tile([128, F], bf16, tag=f"S{g}",
                                        name=f"st{g}_{s}")
                    nc.vector.tensor_mul(sn_new[:], ps[:],
                                         R[w][:, si, g * F:(g + 1) * F])
                    S[g] = sn_new
                    if s == W - 1:
                        nc.vector.tensor_copy(SN[g][:], S[g][:])

            for g in range(G):
                nc.sync.dma_start(snap_out[:, g * F:(g + 1) * F], SN[g][:])
                nc.sync.dma_start(fin_out[:, g * F:(g + 1) * F], S[g][:])

    nc.compile()
    return nc


def _get_nc():
    if "nc" not in _cache:
        _cache["nc"] = _build()
    return _cache["nc"]


def _log_softmax64(v, axis):
    v = v.astype(np.float64)
    m = v.max(axis=axis, keepdims=True)
    e = np.exp(v - m)
    return v - m - np.log(e.sum(axis=axis, keepdims=True))


def _estimate_delta(log_pdf, T64):
    # E[log c] from a vectorized short scan: 64 parallel probes, 56 steps,
    # burn-in 16 (mixing time is ~10 steps).
    NCH, NST, BURN = 64, 56, 16
    cols = np.arange(NCH) * 997 + 1
    a = np.full((K, NCH), 1.0 / K)
    samples = []
    for s in range(NST):
        p = np.exp(log_pdf[:, cols + s].astype(np.float64))
        a = p * (T64 @ a)
        c = a.sum(axis=0)
        a /= c
        if s >= BURN:
            samples.append(np.log(c))
    return float(np.mean(samples))


def _make_in_maps(log_pdf, T64):
    from ml_dtypes import bfloat16

    T32 = T64.astype(np.float32)
    Tbf = T32.astype(bfloat16)
    delta = _estimate_delta(log_pdf, T64)
    # bf16-quantized T is exactly D_r @ T_hat with T_hat row-stochastic and
    # r the bf16 row sums; fold -log(r) and the drift -delta into the exp.
    r = Tbf.astype(np.float64).sum(axis=1)
    eb = np.zeros((128, 1), dtype=np.float32)
    for q in range(4):
        eb[32 * q:32 * q + 32, 0] = (-np.log(r) - delta).astype(np.float32)
    wm = np.zeros((128, 128), dtype=bfloat16)
    for q in range(4):
        wm[32 * q:32 * q + 32, 32 * q:32 * q + 32] = Tbf.T

    # Pack per-core inputs into the tile layout [128, SPAN, NB]:
    # x[32q+k, s, cb] = log_pdf[k, c0 + (4*cb+q)*L + s]
    cols = (np.arange(CC) * L)[None, :] + np.arange(SPAN)[:, None]  # [SPAN, CC]
    in_maps = []
    for k in range(NCORES):
        c0 = k * CC * L
        lp_loc = log_pdf[:, c0:c0 + CC * L + W]
        A = lp_loc[:, cols]                      # [32, SPAN, CC]
        A = A.reshape(32, SPAN, NB, 4)           # chain = 4*cb + q
        A = A.transpose(3, 0, 1, 2)              # [q, 32, SPAN, NB]
        xp = np.ascontiguousarray(
            A.reshape(128, SPAN * NB), dtype=np.float32)
        in_maps.append({"x": xp, "wmat": wm, "ebias": eb})

    return in_maps, delta


def kernel(log_pdf: np.ndarray, pi: np.ndarray, T: np.ndarray) -> np.ndarray:
    from concourse.bass_utils import run_bass_kernel_spmd

    log_pdf = np.ascontiguousarray(log_pdf, dtype=np.float32)
    log_pi64 = _log_softmax64(pi, 0)
    log_T64 = _log_softmax64(T, 1)
    T64 = np.exp(log_T64)                     # row-stochastic [K, K] f64

    in_maps, delta = _make_in_maps(log_pdf, T64)
    nc = _get_nc()
    res = run_bass_kernel_spmd(nc, in_maps, list(range(NCORES))).results

    # ---- host combine (f64) ----
    LP = log_pdf
    # exact prefix [0, W)
    a = np.exp(log_pi64 + LP[:, 0].astype(np.float64))
    c = a.sum()
    total = np.log(c)
    a /= c
    for t in range(1, W):
        a = np.exp(LP[:, t].astype(np.float64)) * (T64 @ a)
        c = a.sum()
        total += np.log(c)
        a /= c

    # per-chain contributions: log(sum fin) - log(sum snap) + delta*L
    for k in range(NCORES):
        snap = res[k]["snap_out"].astype(np.float64)   # [128, NB]
        fin = res[k]["fin_out"].astype(np.float64)
        for q in range(4):
            ssum = snap[32 * q:32 * q + 32, :].sum(axis=0)
            fsum = fin[32 * q:32 * q + 32, :].sum(axis=0)
            total += (np.log(fsum) - np.log(ssum)).sum() + delta * L * NB

    # exact tail [COVERED, N) from the last chain's final state
    k, cb, q = NCORES - 1, NB - 1, 3
    fv = res[k]["fin_out"][32 * q:32 * q + 32, cb].astype(np.float64)
    a = fv / fv.sum()
    for t in range(COVERED, N):
        a = np.exp(LP[:, t].astype(np.float64)) * (T64 @ a)
        c = a.sum()
        total += np.log(c)
        a /= c

    return np.float32(total)


# revision 5
# speedup vs baseline: 3.1740x; 1.0447x over previous
"""HMM log-likelihood (log-domain forward algorithm) on 8 Trainium2 cores.

Strategy: scaled linear-domain forward algorithm with warmup-halo sequence
parallelism.  The filtering distribution of an HMM forgets its initial
condition geometrically fast (|lambda_2(T)| ~ 0.24), so N=1e6 timesteps are
split into short independent chains; each chain starts from a uniform state
W steps before its owned region of L steps.  Per core, chains are batched
4-wide across the 128 SBUF partitions (block-diagonal T^T weights on the PE)
with the chain-block index in the matmul free dimension, so one timestep is
G matmuls (T @ S into PSUM) plus G vector multiplies by the emission
probabilities.

The host pre-packs the per-core input into the exact SBUF tile layout
[128 partitions, SPAN, NB] (time-major), so every DMA is a fully
contiguous [128, chunk] copy at near-peak HBM bandwidth, and every
per-step emission slice is a contiguous [128, F] access.

Normalization is free: a constant per-step drift delta = E[log c] is folded
into the exp bias, making log|S| a zero-drift random walk, so the kernel
needs no per-chain rescaling.  The bf16 quantization of T factors exactly as
D_r @ T_hat with T_hat row-stochastic; -log(r) is folded into the same exp
bias.  Each chain's contribution is log(sum(S_final)) - log(sum(S_at_W)) +
delta*L, assembled on the host, which also runs exact f64 scans for the
prefix [0, W) and the short tail.
"""

import sys

for p in ("/opt/trn_rl_repo", "/root/.axon_site", "/root/.axon_site/_ro/trn_rl_repo",
          "/root/.axon_site/_ro/pypackages"):
    if p not in sys.path:
        sys.path.insert(0, p)

import numpy as np

K = 32
N = 1_000_000
NCORES = 8
W = 2             # warmup (halo) steps per chain (|lambda_2| ~ 0.24)
L = 31            # owned steps per chain
SPAN = W + L      # 33 sequential steps
CC = 4032         # chains per core (owned columns = CC*L = 124992)
NB = CC // 4      # 1008 four-chain blocks (matmul/mul free dim)
G = 2             # interleaved compute streams
F = NB // G       # 504 blocks per stream (fits one PSUM bank in f32)
# window sizes (timesteps per DMA): small first windows so compute starts
# early, small last windows so the post-DMA tail is short
WSCHED = [1, 1, 2, 3, 4, 5, 6, 6, 3, 2]
assert sum(WSCHED) == SPAN
NWIN = len(WSCHED)
WOFF = [sum(WSCHED[:i]) for i in range(NWIN)]
COVERED = W + NCORES * CC * L

_cache = {}


def _build():
    import concourse.bass as bass
    import concourse.bacc as bacc
    import concourse.mybir as mybir
    import concourse.tile as tile
    from contextlib import ExitStack

    f32 = mybir.dt.float32
    bf16 = mybir.dt.bfloat16
    AF = mybir.ActivationFunctionType

    M = SPAN * NB  # per-partition input columns

    nc = bacc.Bacc("TRN2", target_bir_lowering=False, debug=False,
                   num_devices=NCORES)
    x = nc.dram_tensor("x", [128, M], f32, kind="ExternalInput")
    wmat = nc.dram_tensor("wmat", [128, 128], bf16, kind="ExternalInput")
    ebias = nc.dram_tensor("ebias", [128, 1], f32, kind="ExternalInput")
    snap_out = nc.dram_tensor("snap_out", [128, NB], bf16, kind="ExternalOutput")
    fin_out = nc.dram_tensor("fin_out", [128, NB], bf16, kind="ExternalOutput")

    with tile.TileContext(nc) as tc:
        with ExitStack() as ctx:
            cpool = ctx.enter_context(tc.tile_pool(name="const", bufs=1))
            rpool = ctx.enter_context(tc.tile_pool(name="rp", bufs=3))
            pspool = ctx.enter_context(
                tc.tile_pool(name="ps", bufs=4, space=bass.MemorySpace.PSUM))

            w_t = cpool.tile([128, 128], bf16, tag="w")
            nc.sync.dma_start(w_t[:], wmat[:])
            eb_t = cpool.tile([128, 1], f32, tag="eb")
            nc.sync.dma_start(eb_t[:], ebias[:])

            spool = ctx.enter_context(tc.tile_pool(name="sp", bufs=2))
            S, SN = [], []
            for g in range(G):
                st = spool.tile([128, F], bf16, tag=f"S{g}", name=f"st{g}")
                nc.vector.memset(st[:], 1.0)
                sn = cpool.tile([128, F], bf16, tag=f"N{g}")
                S.append(st)
                SN.append(sn)

            # PE warm-up: back-to-back dummy matmuls while the first input
            # window is still in flight, so the HAM clock-gate opens
            # (1.2 -> 2.4 GHz) before the real step loop begins.
            for _ in range(8):
                warm_ps = pspool.tile([128, F], f32, tag="mm0",
                                      name="warm")
                nc.tensor.matmul(warm_ps[:], w_t[:], S[0][:],
                                 start=True, stop=True)

            # Load + exp windows.  R[w] layout: [128, sblk, NB]; partition
            # p = 32*q + k holds chain 4*cb + q, state k at free (s, cb).
            # The host packs x so each window is one contiguous DMA.
            R = [None] * NWIN
            for w in range(NWIN):
                sblk = WSCHED[w]
                rt = rpool.tile([128, sblk, NB], f32, tag="R", name=f"rt{w}")
                src = bass.AP(x, WOFF[w] * NB,
                              [[M, 128], [NB, sblk], [1, NB]])
                nc.sync.dma_start(rt[:], src)
                R[w] = rt
                # exp in chunks of <=3 timesteps for finer compute overlap
                s0 = 0
                while s0 < sblk:
                    cs = min(3, sblk - s0)
                    nc.scalar.activation(
                        rt[:, s0:s0 + cs, :], rt[:, s0:s0 + cs, :], AF.Exp,
                        bias=eb_t[:])
                    s0 += cs

            step_win = []
            for w in range(NWIN):
                step_win += [(w, si) for si in range(WSCHED[w])]
            for s in range(SPAN):
                w, si = step_win[s]
                for g in range(G):
                    ps = pspool.tile([128, F], f32, tag=f"mm{g}")
                    nc.tensor.matmul(ps[:], w_t[:], S[g][:], start=True, stop=True)
                    # ping-pong the state tile so the multiply never WARs
                    # against this step's matmul read
                    sn_new = spool.tile([128, F], bf16, tag=f"S{g}",
                                        name=f"st{g}_{s}")
                    nc.vector.tensor_mul(sn_new[:], ps[:],
                                         R[w][:, si, g * F:(g + 1) * F])
                    S[g] = sn_new
                    if s == W - 1:
                        nc.vector.tensor_copy(SN[g][:], S[g][:])

            for g in range(G):
                nc.sync.dma_start(snap_out[:, g * F:(g + 1) * F], SN[g][:])
                nc.sync.dma_start(fin_out[:, g * F:(g + 1) * F], S[g][:])

    nc.compile()
    return nc


def _get_nc():
    if "nc" not in _cache:
        _cache["nc"] = _build()
    return _cache["nc"]


def _log_softmax64(v, axis):
    v = v.astype(np.float64)
    m = v.max(axis=axis, keepdims=True)
    e = np.exp(v - m)
    return v - m - np.log(e.sum(axis=axis, keepdims=True))


def _estimate_delta(log_pdf, T64):
    # E[log c] from a vectorized short scan: 64 parallel probes, 56 steps,
    # burn-in 16 (mixing time is ~10 steps).
    NCH, NST, BURN = 64, 56, 16
    cols = np.arange(NCH) * 997 + 1
    a = np.full((K, NCH), 1.0 / K)
    samples = []
    for s in range(NST):
        p = np.exp(log_pdf[:, cols + s].astype(np.float64))
        a = p * (T64 @ a)
        c = a.sum(axis=0)
        a /= c
        if s >= BURN:
            samples.append(np.log(c))
    return float(np.mean(samples))


def _make_in_maps(log_pdf, T64):
    from ml_dtypes import bfloat16

    T32 = T64.astype(np.float32)
    Tbf = T32.astype(bfloat16)
    delta = _estimate_delta(log_pdf, T64)
    # bf16-quantized T is exactly D_r @ T_hat with T_hat row-stochastic and
    # r the bf16 row sums; fold -log(r) and the drift -delta into the exp.
    r = Tbf.astype(np.float64).sum(axis=1)
    eb = np.zeros((128, 1), dtype=np.float32)
    for q in range(4):
        eb[32 * q:32 * q + 32, 0] = (-np.log(r) - delta).astype(np.float32)
    wm = np.zeros((128, 128), dtype=bfloat16)
    for q in range(4):
        wm[32 * q:32 * q + 32, 32 * q:32 * q + 32] = Tbf.T

    # Pack per-core inputs into the tile layout [128, SPAN, NB]:
    # x[32q+k, s, cb] = log_pdf[k, c0 + (4*cb+q)*L + s]
    cols = (np.arange(CC) * L)[None, :] + np.arange(SPAN)[:, None]  # [SPAN, CC]
    in_maps = []
    for k in range(NCORES):
        c0 = k * CC * L
        lp_loc = log_pdf[:, c0:c0 + CC * L + W]
        A = lp_loc[:, cols]                      # [32, SPAN, CC]
        A = A.reshape(32, SPAN, NB, 4)           # chain = 4*cb + q
        A = A.transpose(3, 0, 1, 2)              # [q, 32, SPAN, NB]
        xp = np.ascontiguousarray(
            A.reshape(128, SPAN * NB), dtype=np.float32)
        in_maps.append({"x": xp, "wmat": wm, "ebias": eb})

    return in_maps, delta


def kernel(log_pdf: np.ndarray, pi: np.ndarray, T: np.ndarray) -> np.ndarray:
    from concourse.bass_utils import run_bass_kernel_spmd

    log_pdf = np.ascontiguousarray(log_pdf, dtype=np.float32)
    log_pi64 = _log_softmax64(pi, 0)
    log_T64 = _log_softmax64(T, 1)
    T64 = np.exp(log_T64)                     # row-stochastic [K, K] f64

    in_maps, delta = _make_in_maps(log_pdf, T64)
    nc = _get_nc()
    res = run_bass_kernel_spmd(nc, in_maps, list(range(NCORES))).results

    # ---- host combine (f64) ----
    LP = log_pdf
    # exact prefix [0, W)
    a = np.exp(log_pi64 + LP[:, 0].astype(np.float64))
    c = a.sum()
    total = np.log(c)
    a /= c
    for t in range(1, W):
        a = np.exp(LP[:, t].astype(np.float64)) * (T64 @ a)
        c = a.sum()
        total += np.log(c)
        a /= c

    # per-chain contributions: log(sum fin) - log(sum snap) + delta*L
    for k in range(NCORES):
        snap = res[k]["snap_out"].astype(np.float64)   # [128, NB]
        fin = res[k]["fin_out"].astype(np.float64)
        for q in range(4):
            ssum = snap[32 * q:32 * q + 32, :].sum(axis=0)
            fsum = fin[32 * q:32 * q + 32, :].sum(axis=0)
            total += (np.log(fsum) - np.log(ssum)).sum() + delta * L * NB

    # exact tail [COVERED, N) from the last chain's final state
    k, cb, q = NCORES - 1, NB - 1, 3
    fv = res[k]["fin_out"][32 * q:32 * q + 32, cb].astype(np.float64)
    a = fv / fv.sum()
    for t in range(COVERED, N):
        a = np.exp(LP[:, t].astype(np.float64)) * (T64 @ a)
        c = a.sum()
        total += np.log(c)
        a /= c

    return np.float32(total)


# revision 9
# speedup vs baseline: 3.1896x; 1.0049x over previous
"""HMM log-likelihood (log-domain forward algorithm) on 8 Trainium2 cores.

Strategy: scaled linear-domain forward algorithm with warmup-halo sequence
parallelism.  The filtering distribution of an HMM forgets its initial
condition geometrically fast (|lambda_2(T)| ~ 0.24), so N=1e6 timesteps are
split into short independent chains; each chain starts from a uniform state
W steps before its owned region of L steps.  Per core, chains are batched
4-wide across the 128 SBUF partitions (block-diagonal T^T weights on the PE)
with the chain-block index in the matmul free dimension, so one timestep is
G matmuls (T @ S into PSUM) plus G vector multiplies by the emission
probabilities.

The host pre-packs the per-core input into the exact SBUF tile layout
[128 partitions, SPAN, NB] (time-major), so every DMA is a fully
contiguous [128, chunk] copy at near-peak HBM bandwidth, and every
per-step emission slice is a contiguous [128, F] access.

Normalization is free: a constant per-step drift delta = E[log c] is folded
into the exp bias, making log|S| a zero-drift random walk, so the kernel
needs no per-chain rescaling.  The bf16 quantization of T factors exactly as
D_r @ T_hat with T_hat row-stochastic; -log(r) is folded into the same exp
bias.  Each chain's contribution is log(sum(S_final)) - log(sum(S_at_W)) +
delta*L, assembled on the host, which also runs exact f64 scans for the
prefix [0, W) and the short tail.
"""

import sys

for p in ("/opt/trn_rl_repo", "/root/.axon_site", "/root/.axon_site/_ro/trn_rl_repo",
          "/root/.axon_site/_ro/pypackages"):
    if p not in sys.path:
        sys.path.insert(0, p)

import numpy as np

K = 32
N = 1_000_000
NCORES = 8
W = 2             # warmup (halo) steps per chain (|lambda_2| ~ 0.24)
L = 31            # owned steps per chain
SPAN = W + L      # 33 sequential steps
CC = 4032         # chains per core (owned columns = CC*L = 124992)
NB = CC // 4      # 1008 four-chain blocks (matmul/mul free dim)
G = 2             # interleaved compute streams
F = NB // G       # 504 blocks per stream (fits one PSUM bank in f32)
# window sizes (timesteps per DMA): small first windows so compute starts
# early, small last windows so the post-DMA tail is short
WSCHED = [1, 1, 2, 2, 3, 4, 5, 6, 6, 2, 1]
assert sum(WSCHED) == SPAN
NWIN = len(WSCHED)
WOFF = [sum(WSCHED[:i]) for i in range(NWIN)]
COVERED = W + NCORES * CC * L

_cache = {}


def _build():
    import concourse.bass as bass
    import concourse.bacc as bacc
    import concourse.mybir as mybir
    import concourse.tile as tile
    from contextlib import ExitStack

    f32 = mybir.dt.float32
    bf16 = mybir.dt.bfloat16
    AF = mybir.ActivationFunctionType

    M = SPAN * NB  # per-partition input columns

    nc = bacc.Bacc("TRN2", target_bir_lowering=False, debug=False,
                   num_devices=NCORES)
    x = nc.dram_tensor("x", [128, M], f32, kind="ExternalInput")
    wmat = nc.dram_tensor("wmat", [128, 128], bf16, kind="ExternalInput")
    ebias = nc.dram_tensor("ebias", [128, 1], f32, kind="ExternalInput")
    snap_out = nc.dram_tensor("snap_out", [128, NB], bf16, kind="ExternalOutput")
    fin_out = nc.dram_tensor("fin_out", [128, NB], bf16, kind="ExternalOutput")

    with tile.TileContext(nc) as tc:
        with ExitStack() as ctx:
            cpool = ctx.enter_context(tc.tile_pool(name="const", bufs=1))
            rpool = ctx.enter_context(tc.tile_pool(name="rp", bufs=4))
            pspool = ctx.enter_context(
                tc.tile_pool(name="ps", bufs=3, space=bass.MemorySpace.PSUM))
            wpool = ctx.enter_context(
                tc.tile_pool(name="wp", bufs=2, space=bass.MemorySpace.PSUM))

            # issue the first input window's DMA before anything else so
            # compute can start as early as possible
            rpre = rpool.tile([128, WSCHED[0], NB], f32, tag="R", name="rt0")
            src0 = bass.AP(x, 0, [[M, 128], [NB, WSCHED[0]], [1, NB]])
            nc.sync.dma_start(rpre[:], src0)
            eb_t = cpool.tile([128, 1], f32, tag="eb")
            nc.sync.dma_start(eb_t[:], ebias[:])
            w_t = cpool.tile([128, 128], bf16, tag="w")
            nc.sync.dma_start(w_t[:], wmat[:])

            spool = ctx.enter_context(tc.tile_pool(name="sp", bufs=2))
            S, SN = [], []
            for g in range(G):
                st = spool.tile([128, F], bf16, tag=f"S{g}", name=f"st{g}")
                nc.gpsimd.memset(st[:], 1.0)
                sn = cpool.tile([128, F], bf16, tag=f"N{g}")
                S.append(st)
                SN.append(sn)

            # PE warm-up: back-to-back dummy matmuls while the first input
            # window is still in flight, so the HAM clock-gate opens
            # (1.2 -> 2.4 GHz) before the real step loop begins.
            def filler(n, fd):
                for _ in range(n):
                    wps = wpool.tile([128, fd], f32, tag="warm", name="warm")
                    nc.tensor.matmul(wps[:], w_t[:], w_t[:, :fd],
                                     start=True, stop=True)

            filler(8, 128)

            # Load + exp windows.  R[w] layout: [128, sblk, NB]; partition
            # p = 32*q + k holds chain 4*cb + q, state k at free (s, cb).
            # The host packs x so each window is one contiguous DMA.
            R = [None] * NWIN
            for w in range(NWIN):
                sblk = WSCHED[w]
                if w == 0:
                    rt = rpre
                else:
                    rt = rpool.tile([128, sblk, NB], f32, tag="R",
                                    name=f"rt{w}")
                    src = bass.AP(x, WOFF[w] * NB,
                                  [[M, 128], [NB, sblk], [1, NB]])
                    nc.sync.dma_start(rt[:], src)
                R[w] = rt
                # exp in chunks of <=3 timesteps for finer compute overlap
                s0 = 0
                while s0 < sblk:
                    cs = min(3, sblk - s0)
                    nc.scalar.activation(
                        rt[:, s0:s0 + cs, :], rt[:, s0:s0 + cs, :], AF.Exp,
                        bias=eb_t[:])
                    s0 += cs

            step_win = []
            for w in range(NWIN):
                step_win += [(w, si) for si in range(WSCHED[w])]
            for s in range(SPAN):
                w, si = step_win[s]
                for g in range(G):
                    ps = pspool.tile([128, F], f32, tag=f"mm{g}")
                    nc.tensor.matmul(ps[:], w_t[:], S[g][:], start=True, stop=True)
                    # ping-pong the state tile so the multiply never WARs
                    # against this step's matmul read
                    sn_new = spool.tile([128, F], bf16, tag=f"S{g}",
                                        name=f"st{g}_{s}")
                    nc.vector.tensor_mul(sn_new[:], ps[:],
                                         R[w][:, si, g * F:(g + 1) * F])
                    S[g] = sn_new
                    if s == W - 1:
                        # snapshot on the scalar engine (DVE is the
                        # bottleneck) and ship it to HBM immediately
                        nc.scalar.copy(SN[g][:], S[g][:])
                        nc.sync.dma_start(snap_out[:, g * F:(g + 1) * F],
                                          SN[g][:])
                # keep the PE's HAM activity monitor above the un-throttle
                # threshold: dep-free filler matmuls absorb the idle gaps
                # (ramp stalls would otherwise re-throttle the clock to
                # 1.2 GHz for the whole kernel)
                filler(3 if s < 8 else 1, 128)

            for g in range(G):
                nc.sync.dma_start(fin_out[:, g * F:(g + 1) * F], S[g][:])

    nc.compile()
    return nc


def _get_nc():
    if "nc" not in _cache:
        _cache["nc"] = _build()
    return _cache["nc"]


def _log_softmax64(v, axis):
    v = v.astype(np.float64)
    m = v.max(axis=axis, keepdims=True)
    e = np.exp(v - m)
    return v - m - np.log(e.sum(axis=axis, keepdims=True))


def _estimate_delta(log_pdf, T64):
    # E[log c] from a vectorized short scan: 64 parallel probes, 56 steps,
    # burn-in 16 (mixing time is ~10 steps).
    NCH, NST, BURN = 64, 56, 16
    cols = np.arange(NCH) * 997 + 1
    a = np.full((K, NCH), 1.0 / K)
    samples = []
    for s in range(NST):
        p = np.exp(log_pdf[:, cols + s].astype(np.float64))
        a = p * (T64 @ a)
        c = a.sum(axis=0)
        a /= c
        if s >= BURN:
            samples.append(np.log(c))
    return float(np.mean(samples))


def _make_in_maps(log_pdf, T64):
    from ml_dtypes import bfloat16

    T32 = T64.astype(np.float32)
    Tbf = T32.astype(bfloat16)
    delta = _estimate_delta(log_pdf, T64)
    # bf16-quantized T is exactly D_r @ T_hat with T_hat row-stochastic and
    # r the bf16 row sums; fold -log(r) and the drift -delta into the exp.
    r = Tbf.astype(np.float64).sum(axis=1)
    eb = np.zeros((128, 1), dtype=np.float32)
    for q in range(4):
        eb[32 * q:32 * q + 32, 0] = (-np.log(r) - delta).astype(np.float32)
    wm = np.zeros((128, 128), dtype=bfloat16)
    for q in range(4):
        wm[32 * q:32 * q + 32, 32 * q:32 * q + 32] = Tbf.T

    # Pack per-core inputs into the tile layout [128, SPAN, NB]:
    # x[32q+k, s, cb] = log_pdf[k, c0 + (4*cb+q)*L + s]
    cols = (np.arange(CC) * L)[None, :] + np.arange(SPAN)[:, None]  # [SPAN, CC]
    in_maps = []
    for k in range(NCORES):
        c0 = k * CC * L
        lp_loc = log_pdf[:, c0:c0 + CC * L + W]
        A = lp_loc[:, cols]                      # [32, SPAN, CC]
        A = A.reshape(32, SPAN, NB, 4)           # chain = 4*cb + q
        A = A.transpose(3, 0, 1, 2)              # [q, 32, SPAN, NB]
        xp = np.ascontiguousarray(
            A.reshape(128, SPAN * NB), dtype=np.float32)
        in_maps.append({"x": xp, "wmat": wm, "ebias": eb})

    return in_maps, delta


def kernel(log_pdf: np.ndarray, pi: np.ndarray, T: np.ndarray) -> np.ndarray:
    from concourse.bass_utils import run_bass_kernel_spmd

    log_pdf = np.ascontiguousarray(log_pdf, dtype=np.float32)
    log_pi64 = _log_softmax64(pi, 0)
    log_T64 = _log_softmax64(T, 1)
    T64 = np.exp(log_T64)                     # row-stochastic [K, K] f64

    in_maps, delta = _make_in_maps(log_pdf, T64)
    nc = _get_nc()
    res = run_bass_kernel_spmd(nc, in_maps, list(range(NCORES))).results

    # ---- host combine (f64) ----
    LP = log_pdf
    # exact prefix [0, W)
    a = np.exp(log_pi64 + LP[:, 0].astype(np.float64))
    c = a.sum()
    total = np.log(c)
    a /= c
    for t in range(1, W):
        a = np.exp(LP[:, t].astype(np.float64)) * (T64 @ a)
        c = a.sum()
        total += np.log(c)
        a /= c

    # per-chain contributions: log(sum fin) - log(sum snap) + delta*L
    for k in range(NCORES):
        snap = res[k]["snap_out"].astype(np.float64)   # [128, NB]
        fin = res[k]["fin_out"].astype(np.float64)
        for q in range(4):
            ssum = snap[32 * q:32 * q + 32, :].sum(axis=0)
            fsum = fin[32 * q:32 * q + 32, :].sum(axis=0)
            total += (np.log(fsum) - np.log(ssum)).sum() + delta * L * NB

    # exact tail [COVERED, N) from the last chain's final state
    k, cb, q = NCORES - 1, NB - 1, 3
    fv = res[k]["fin_out"][32 * q:32 * q + 32, cb].astype(np.float64)
    a = fv / fv.sum()
    for t in range(COVERED, N):
        a = np.exp(LP[:, t].astype(np.float64)) * (T64 @ a)
        c = a.sum()
        total += np.log(c)
        a /= c

    return np.float32(total)


# revision 14
# speedup vs baseline: 3.3368x; 1.0462x over previous
"""HMM log-likelihood (log-domain forward algorithm) on 8 Trainium2 cores.

Strategy: scaled linear-domain forward algorithm with warmup-halo sequence
parallelism.  The filtering distribution of an HMM forgets its initial
condition geometrically fast (|lambda_2(T)| ~ 0.24), so N=1e6 timesteps are
split into short independent chains; each chain starts from a uniform state
W steps before its owned region of L steps.  Per core, chains are batched
4-wide across the 128 SBUF partitions (block-diagonal T^T weights on the PE)
with the chain-block index in the matmul free dimension, so one timestep is
G matmuls (T @ S into PSUM) plus G vector multiplies by the emission
probabilities.

The host pre-packs the per-core input into the exact SBUF tile layout
[128 partitions, SPAN, NB] (time-major), so every DMA is a fully
contiguous [128, chunk] copy at near-peak HBM bandwidth, and every
per-step emission slice is a contiguous [128, F] access.

Normalization is free: a constant per-step drift delta = E[log c] is folded
into the exp bias, making log|S| a zero-drift random walk, so the kernel
needs no per-chain rescaling.  The bf16 quantization of T factors exactly as
D_r @ T_hat with T_hat row-stochastic; -log(r) is folded into the same exp
bias.  Each chain's contribution is log(sum(S_final)) - log(sum(S_at_W)) +
delta*L, assembled on the host, which also runs exact f64 scans for the
prefix [0, W) and the short tail.
"""

import sys

for p in ("/opt/trn_rl_repo", "/root/.axon_site", "/root/.axon_site/_ro/trn_rl_repo",
          "/root/.axon_site/_ro/pypackages"):
    if p not in sys.path:
        sys.path.insert(0, p)

import numpy as np

K = 32
N = 1_000_000
NCORES = 8
W = 2             # warmup (halo) steps per chain (|lambda_2| ~ 0.24)
L = 31            # owned steps per chain
SPAN = W + L      # 33 sequential steps
CC = 4032         # chains per core (owned columns = CC*L = 124992)
NB = CC // 4      # 1008 four-chain blocks (matmul/mul free dim)
G = 2             # interleaved compute streams
F = NB // G       # 504 blocks per stream (fits one PSUM bank in f32)
# window sizes (timesteps per DMA): small first windows so compute starts
# early, small last windows so the post-DMA tail is short
WSCHED = [1, 1, 2, 2, 3, 4, 4, 4, 4, 4, 2, 1, 1]
assert sum(WSCHED) == SPAN
NWIN = len(WSCHED)
WOFF = [sum(WSCHED[:i]) for i in range(NWIN)]
COVERED = W + NCORES * CC * L

_cache = {}


def _build():
    import concourse.bass as bass
    import concourse.bacc as bacc
    import concourse.mybir as mybir
    import concourse.tile as tile
    from contextlib import ExitStack

    f32 = mybir.dt.float32
    bf16 = mybir.dt.bfloat16
    AF = mybir.ActivationFunctionType

    M = SPAN * NB  # per-partition input columns

    nc = bacc.Bacc("TRN2", target_bir_lowering=False, debug=False,
                   num_devices=NCORES)
    x = nc.dram_tensor("x", [128, M], f32, kind="ExternalInput")
    wmat = nc.dram_tensor("wmat", [128, 128], bf16, kind="ExternalInput")
    ebias = nc.dram_tensor("ebias", [128, 1], f32, kind="ExternalInput")
    snap_out = nc.dram_tensor("snap_out", [128, NB], bf16, kind="ExternalOutput")
    fin_out = nc.dram_tensor("fin_out", [128, NB], bf16, kind="ExternalOutput")

    with tile.TileContext(nc) as tc:
        with ExitStack() as ctx:
            cpool = ctx.enter_context(tc.tile_pool(name="const", bufs=1))
            rpool = ctx.enter_context(tc.tile_pool(name="rp", bufs=8))
            pspool = ctx.enter_context(
                tc.tile_pool(name="ps", bufs=4, space=bass.MemorySpace.PSUM))

            # issue the first input window's DMA before anything else so
            # compute can start as early as possible
            rpre = rpool.tile([128, WSCHED[0], NB], f32, tag="R", name="rt0")
            src0 = bass.AP(x, 0, [[M, 128], [NB, WSCHED[0]], [1, NB]])
            nc.sync.dma_start(rpre[:], src0)
            eb_t = cpool.tile([128, 1], f32, tag="eb")
            nc.sync.dma_start(eb_t[:], ebias[:])
            w_t = cpool.tile([128, 128], bf16, tag="w")
            nc.sync.dma_start(w_t[:], wmat[:])

            spool = ctx.enter_context(tc.tile_pool(name="sp", bufs=2))
            S, SN = [], []
            for g in range(G):
                st = spool.tile([128, F], bf16, tag=f"S{g}", name=f"st{g}")
                nc.gpsimd.memset(st[:], 1.0)
                sn = cpool.tile([128, F], bf16, tag=f"N{g}")
                S.append(st)
                SN.append(sn)

            # Load + exp windows.  R[w] layout: [128, sblk, NB]; partition
            # p = 32*q + k holds chain 4*cb + q, state k at free (s, cb).
            # The host packs x so each window is one contiguous DMA.
            R = [None] * NWIN
            for w in range(NWIN):
                sblk = WSCHED[w]
                if w == 0:
                    rt = rpre
                else:
                    rt = rpool.tile([128, sblk, NB], f32, tag="R",
                                    name=f"rt{w}")
                    src = bass.AP(x, WOFF[w] * NB,
                                  [[M, 128], [NB, sblk], [1, NB]])
                    nc.sync.dma_start(rt[:], src)
                R[w] = rt
                # exp in chunks of <=2 timesteps for finer compute overlap
                s0 = 0
                while s0 < sblk:
                    cs = min(2, sblk - s0)
                    nc.scalar.activation(
                        rt[:, s0:s0 + cs, :], rt[:, s0:s0 + cs, :], AF.Exp,
                        bias=eb_t[:])
                    s0 += cs

            step_win = []
            for w in range(NWIN):
                step_win += [(w, si) for si in range(WSCHED[w])]
            for s in range(SPAN):
                w, si = step_win[s]
                for g in range(G):
                    ps = pspool.tile([128, F], f32, tag=f"mm{g}")
                    nc.tensor.matmul(ps[:], w_t[:], S[g][:], start=True, stop=True)
                    # ping-pong the state tile so the multiply never WARs
                    # against this step's matmul read
                    sn_new = spool.tile([128, F], bf16, tag=f"S{g}",
                                        name=f"st{g}_{s}")
                    nc.vector.tensor_mul(sn_new[:], ps[:],
                                         R[w][:, si, g * F:(g + 1) * F])
                    S[g] = sn_new
                    if s == W - 1:
                        # snapshot on the scalar engine (DVE is the
                        # bottleneck) and ship it to HBM immediately
                        nc.scalar.copy(SN[g][:], S[g][:])
                        nc.sync.dma_start(snap_out[:, g * F:(g + 1) * F],
                                          SN[g][:])
            for g in range(G):
                nc.sync.dma_start(fin_out[:, g * F:(g + 1) * F], S[g][:])

    nc.compile()
    return nc


def _get_nc():
    if "nc" not in _cache:
        _cache["nc"] = _build()
    return _cache["nc"]


def _log_softmax64(v, axis):
    v = v.astype(np.float64)
    m = v.max(axis=axis, keepdims=True)
    e = np.exp(v - m)
    return v - m - np.log(e.sum(axis=axis, keepdims=True))


def _estimate_delta(log_pdf, T64):
    # E[log c] from a vectorized short scan: 64 parallel probes, 56 steps,
    # burn-in 16 (mixing time is ~10 steps).
    NCH, NST, BURN = 64, 56, 16
    cols = np.arange(NCH) * 997 + 1
    a = np.full((K, NCH), 1.0 / K)
    samples = []
    for s in range(NST):
        p = np.exp(log_pdf[:, cols + s].astype(np.float64))
        a = p * (T64 @ a)
        c = a.sum(axis=0)
        a /= c
        if s >= BURN:
            samples.append(np.log(c))
    return float(np.mean(samples))


def _make_in_maps(log_pdf, T64):
    from ml_dtypes import bfloat16

    T32 = T64.astype(np.float32)
    Tbf = T32.astype(bfloat16)
    delta = _estimate_delta(log_pdf, T64)
    # bf16-quantized T is exactly D_r @ T_hat with T_hat row-stochastic and
    # r the bf16 row sums; fold -log(r) and the drift -delta into the exp.
    r = Tbf.astype(np.float64).sum(axis=1)
    eb = np.zeros((128, 1), dtype=np.float32)
    for q in range(4):
        eb[32 * q:32 * q + 32, 0] = (-np.log(r) - delta).astype(np.float32)
    wm = np.zeros((128, 128), dtype=bfloat16)
    for q in range(4):
        wm[32 * q:32 * q + 32, 32 * q:32 * q + 32] = Tbf.T

    # Pack per-core inputs into the tile layout [128, SPAN, NB]:
    # x[32q+k, s, cb] = log_pdf[k, c0 + (4*cb+q)*L + s]
    cols = (np.arange(CC) * L)[None, :] + np.arange(SPAN)[:, None]  # [SPAN, CC]
    in_maps = []
    for k in range(NCORES):
        c0 = k * CC * L
        lp_loc = log_pdf[:, c0:c0 + CC * L + W]
        A = lp_loc[:, cols]                      # [32, SPAN, CC]
        A = A.reshape(32, SPAN, NB, 4)           # chain = 4*cb + q
        A = A.transpose(3, 0, 1, 2)              # [q, 32, SPAN, NB]
        xp = np.ascontiguousarray(
            A.reshape(128, SPAN * NB), dtype=np.float32)
        in_maps.append({"x": xp, "wmat": wm, "ebias": eb})

    return in_maps, delta


def kernel(log_pdf: np.ndarray, pi: np.ndarray, T: np.ndarray) -> np.ndarray:
    from concourse.bass_utils import run_bass_kernel_spmd

    log_pdf = np.ascontiguousarray(log_pdf, dtype=np.float32)
    log_pi64 = _log_softmax64(pi, 0)
    log_T64 = _log_softmax64(T, 1)
    T64 = np.exp(log_T64)                     # row-stochastic [K, K] f64

    in_maps, delta = _make_in_maps(log_pdf, T64)
    nc = _get_nc()
    res = run_bass_kernel_spmd(nc, in_maps, list(range(NCORES))).results

    # ---- host combine (f64) ----
    LP = log_pdf
    # exact prefix [0, W)
    a = np.exp(log_pi64 + LP[:, 0].astype(np.float64))
    c = a.sum()
    total = np.log(c)
    a /= c
    for t in range(1, W):
        a = np.exp(LP[:, t].astype(np.float64)) * (T64 @ a)
        c = a.sum()
        total += np.log(c)
        a /= c

    # per-chain contributions: log(sum fin) - log(sum snap) + delta*L
    for k in range(NCORES):
        snap = res[k]["snap_out"].astype(np.float64)   # [128, NB]
        fin = res[k]["fin_out"].astype(np.float64)
        for q in range(4):
            ssum = snap[32 * q:32 * q + 32, :].sum(axis=0)
            fsum = fin[32 * q:32 * q + 32, :].sum(axis=0)
            total += (np.log(fsum) - np.log(ssum)).sum() + delta * L * NB

    # exact tail [COVERED, N) from the last chain's final state
    k, cb, q = NCORES - 1, NB - 1, 3
    fv = res[k]["fin_out"][32 * q:32 * q + 32, cb].astype(np.float64)
    a = fv / fv.sum()
    for t in range(COVERED, N):
        a = np.exp(LP[:, t].astype(np.float64)) * (T64 @ a)
        c = a.sum()
        total += np.log(c)
        a /= c

    return np.float32(total)
